# revision 1
# baseline (speedup 1.0000x reference)
"""Trainium2 Bass kernel for nn_DAHH (hypergraph conv + BatchNorm + ReLU).

Sharding: data-parallel over B=4 samples x 2 half-row shards = 8 cores.
Each core owns rows R = [h*1024, (h+1)*1024) of its sample's (2048, 768) node
matrix, finds each row's nearest neighbor (kNN hyperedge construction), and
the pair of cores exchanges nn indices via AllGather. BatchNorm statistics are
combined with an 8-core AllReduce.

Self-contained: hardcodes all shapes; only needs numpy + concourse (bass).
"""

import os
import numpy as np

import concourse.bacc as bacc
import concourse.bass as bass
import concourse.mybir as mybir
import concourse.tile as tile
from concourse import bass_utils
from concourse.bass import IndirectOffsetOnAxis

F32 = mybir.dt.float32
F32R = mybir.dt.float32r
U32 = mybir.dt.uint32

B, C, L, OUT = 4, 768, 2048, 159
P = 128
KT = C // P            # 6 k-tiles
HALF = L // 2          # 1024 rows per core
MT = HALF // P         # 8 m-tiles per core (own rows)
JT = L // P            # 16 j-tiles (all rows)
GW = C + 4             # gather row width (768 xi + sq/2 + pad)
NCH = OUT + 1          # 160: padded channel dim (col 159 = ones/deg)
FLAT = HALF * OUT      # 162816 = 159 * 1024 flat elements per core
NCAND = 3              # nn candidates refined exactly (from fp32r top-8)
BN_EPS = 1e-5
NELEM = float(B * L)   # elements per BN channel across batch

LAST_INFO = {}

_CACHE = {}


class _Done(Exception):
    pass


def _build():
    if "nc" in _CACHE:
        return _CACHE["nc"]
    maxphase = int(os.environ.get("KERNEL_MAXPHASE", "99"))
    subp = int(os.environ.get("KERNEL_SUBP", "9"))
    sub6 = int(os.environ.get("KERNEL_SUB6", "9"))

    nc = bacc.Bacc("TRN2", target_bir_lowering=False, debug=False,
                   num_devices=8)

    # ---- DRAM I/O (per-core contents differ, shapes uniform) ----
    xr_d = nc.dram_tensor("xr", [C, L], F32R, kind="ExternalInput")
    xh_d = nc.dram_tensor("xh", [C, HALF], F32R, kind="ExternalInput")
    x32_d = nc.dram_tensor("x32", [C, L], F32, kind="ExternalInput")
    msqh_d = nc.dram_tensor("msqh", [1, L], F32R, kind="ExternalInput")
    onesr_d = nc.dram_tensor("onesr", [1, P], F32R, kind="ExternalInput")
    gsrc_d = nc.dram_tensor("gsrc", [L, GW], F32, kind="ExternalInput")
    gself_d = nc.dram_tensor("gself", [HALF, GW], F32, kind="ExternalInput")
    th_d = nc.dram_tensor("theta", [C, OUT], F32, kind="ExternalInput")
    gam_d = nc.dram_tensor("gamma", [NCH], F32, kind="ExternalInput")
    bet_d = nc.dram_tensor("beta", [NCH], F32, kind="ExternalInput")
    colidx_d = nc.dram_tensor("colidx", [P, HALF], F32, kind="ExternalInput")
    selfidx_d = nc.dram_tensor("selfidx", [P, MT], U32, kind="ExternalInput")
    b2c_d = nc.dram_tensor("b2c", [OUT, NCH], F32, kind="ExternalInput")
    c2b_d = nc.dram_tensor("c2b", [NCH, OUT], F32, kind="ExternalInput")

    y_d = nc.dram_tensor("y", [OUT, HALF], F32, kind="ExternalOutput")
    nn_out_d = nc.dram_tensor("nn_out", [L, 1], U32, kind="ExternalOutput")

    # DRAM scratch
    xt_dram = nc.dram_tensor("xt_scr", [L, OUT], F32)
    e_dram = nc.dram_tensor("e_scr", [L, NCH], F32)
    nn_half = nc.dram_tensor("nn_half", [HALF, 1], U32)
    nn_full = nc.dram_tensor("nn_full", [L, 1], U32)
    nf_flat = nc.dram_tensor("nf_flat", [FLAT], F32)
    stats_in = nc.dram_tensor("stats_in", [NCH, 2], F32)
    stats_out = nc.dram_tensor("stats_out", [NCH, 2], F32)

    with tile.TileContext(nc) as tc:
        with (
            tc.tile_pool(name="main", bufs=1) as mp,
            tc.tile_pool(name="work", bufs=2) as wp,
        ):
            def _body():
                # ---------- persistent loads ----------
                xr_t = [mp.tile([P, L], F32R, name=f"xr{k}") for k in range(KT)]
                xh_t = [mp.tile([P, HALF], F32R, name=f"xh{k}") for k in range(KT)]
                th_t = [mp.tile([P, OUT], F32, name=f"th{k}") for k in range(KT)]
                for k in range(KT):
                    nc.sync.dma_start(xh_t[k][:], xh_d[k * P:(k + 1) * P, :])
                    nc.sync.dma_start(xr_t[k][:], xr_d[k * P:(k + 1) * P, :])
                    nc.sync.dma_start(th_t[k][:], th_d[k * P:(k + 1) * P, :])
                msqh_t = mp.tile([1, L], F32R, name="msqh_t")
                nc.sync.dma_start(msqh_t[:], msqh_d[:, :])
                onesr_t = mp.tile([1, P], F32R, name="onesr_t")
                nc.sync.dma_start(onesr_t[:], onesr_d[:, :])
                colidx_t = mp.tile([P, HALF], F32, name="colidx_t")
                nc.sync.dma_start(colidx_t[:], colidx_d[:, :])
                selfidx_t = mp.tile([P, MT], U32, name="selfidx_t")
                nc.sync.dma_start(selfidx_t[:], selfidx_d[:, :])

                # ---------- phase 1: Gram (fp32r) + top-8 + exact refine ----------
                with tc.tile_pool(name="gramp", bufs=2, space="PSUM") as gp:
                    for m in range(MT):
                        g_ps = gp.tile([P, L], F32, name="g_ps", tag="g")
                        lhs = [xh_t[k][:, m * P:(m + 1) * P] for k in range(KT)]
                        for k in range(KT):
                            for chk in range(L // 512):
                                nc.tensor.matmul(
                                    g_ps[:, chk * 512:(chk + 1) * 512],
                                    lhsT=lhs[k],
                                    rhs=xr_t[k][:, chk * 512:(chk + 1) * 512],
                                    start=(k == 0), stop=False)
                        for chk in range(L // 512):
                            nc.tensor.matmul(
                                g_ps[:, chk * 512:(chk + 1) * 512],
                                lhsT=onesr_t[:, :],
                                rhs=msqh_t[:, chk * 512:(chk + 1) * 512],
                                start=False, stop=True)

                        mneg = wp.tile([P, L], F32, name="mneg", tag="mneg")
                        nc.scalar.activation(mneg[:], g_ps[:],
                                             mybir.ActivationFunctionType.Copy)

                        mx8 = wp.tile([P, 8], F32, name="mx8", tag="mx8")
                        idx8 = wp.tile([P, 8], U32, name="idx8", tag="idx8")
                        nc.vector.max(out=mx8[:], in_=mneg[:])
                        nc.vector.max_index(out=idx8[:], in_max=mx8[:],
                                            in_values=mneg[:])

                        if maxphase < 2:
                            nc.sync.dma_start(nn_half[m * P:(m + 1) * P, :],
                                              idx8[:, 1:2])
                            continue
                        # exact fp32 refine of candidates 1..NCAND
                        gs = wp.tile([P, GW], F32, name="gs", tag="gs")
                        nc.sync.dma_start(gs[:], gself_d[m * P:(m + 1) * P, :])
                        mc_list = []
                        for c in range(1, NCAND + 1):
                            xg = wp.tile([P, GW], F32, name=f"xg{c}", tag="xg",
                                         bufs=3)
                            nc.gpsimd.indirect_dma_start(
                                out=xg[:], out_offset=None,
                                in_=gsrc_d[:, :],
                                in_offset=IndirectOffsetOnAxis(
                                    ap=idx8[:, c:c + 1], axis=0))
                            if subp < 2:
                                continue
                            junk = wp.tile([P, C], F32, name="junk", tag="junk")
                            mdot = wp.tile([P, 1], F32, name=f"mdot{c}",
                                           tag=f"mdot{c}")
                            nc.vector.scalar_tensor_tensor(
                                out=junk[:], in0=gs[:, 0:C], scalar=1.0,
                                in1=xg[:, 0:C],
                                op0=mybir.AluOpType.mult,
                                op1=mybir.AluOpType.mult,
                                accum_out=mdot[:])
                            if subp < 3:
                                continue
                            mc = wp.tile([P, 1], F32, name=f"mc{c}", tag=f"mc{c}")
                            # mc = dot - sq[j]/2  (exact stage-2 score)
                            nc.vector.scalar_tensor_tensor(
                                out=mc[:], in0=mdot[:], scalar=1.0,
                                in1=xg[:, C:C + 1],
                                op0=mybir.AluOpType.mult,
                                op1=mybir.AluOpType.subtract)
                            mc_list.append(mc)
                        if subp < 4:
                            nc.sync.dma_start(nn_half[m * P:(m + 1) * P, :],
                                              idx8[:, 1:2])
                            continue

                        bestm = wp.tile([P, 1], F32, name="bestm", tag="bestm")
                        besti = wp.tile([P, 1], U32, name="besti", tag="besti")
                        nc.vector.tensor_copy(bestm[:], mc_list[0][:])
                        nc.vector.tensor_copy(besti[:], idx8[:, 1:2])
                        for c in range(2, NCAND + 1):
                            mask = wp.tile([P, 1], U32, name=f"mask{c}",
                                           tag=f"mask{c}")
                            nc.vector.tensor_tensor(
                                out=mask[:], in0=mc_list[c - 1][:], in1=bestm[:],
                                op=mybir.AluOpType.is_gt)
                            nc.vector.copy_predicated(bestm[:], mask[:],
                                                      mc_list[c - 1][:])
                            nc.vector.copy_predicated(besti[:], mask[:],
                                                      idx8[:, c:c + 1])
                        nc.sync.dma_start(nn_half[m * P:(m + 1) * P, :], besti[:])

                # ---------- nn allgather within the sample pair ----------
                if maxphase < 3:
                    nc.sync.dma_start(nn_out_d[0:HALF, :], nn_half[:, :])
                    return
                tc.strict_bb_all_engine_barrier()
                nc.gpsimd.collective_compute(
                    "AllGather", mybir.AluOpType.bypass,
                    replica_groups=[[0, 1], [2, 3], [4, 5], [6, 7]],
                    ins=[nn_half.ap().opt()], outs=[nn_full.ap().opt()])
                tc.strict_bb_all_engine_barrier()
                nc.sync.dma_start(nn_out_d[:, :], nn_full[:, :])

                nnu = [mp.tile([P, 1], U32, name=f"nnu{j}") for j in range(JT)]
                nnf = [mp.tile([P, 1], F32, name=f"nnf{j}") for j in range(JT)]
                for j in range(JT):
                    nc.sync.dma_start(nnu[j][:], nn_full[j * P:(j + 1) * P, :])
                    nc.gpsimd.tensor_copy(nnf[j][:], nnu[j][:])

                if maxphase < 4:
                    return
                # ---------- phase 2: xt = xi @ theta (fp32 exact) ----------
                xt_s = [mp.tile([P, OUT], F32, name=f"xts{j}") for j in range(JT)]
                with tc.tile_pool(name="xtp", bufs=2, space="PSUM") as xp:
                    for j in range(JT):
                        xt_ps = xp.tile([P, OUT], F32, name="xt_ps", tag="xtps")
                        for k in range(KT):
                            x32c = wp.tile([P, P], F32, name="x32c", tag="x32c",
                                           bufs=6)
                            nc.sync.dma_start(
                                x32c[:],
                                x32_d[k * P:(k + 1) * P, j * P:(j + 1) * P])
                            nc.tensor.matmul(
                                xt_ps[:], lhsT=x32c[:],
                                rhs=th_t[k][:], start=(k == 0), stop=(k == KT - 1))
                        nc.scalar.activation(xt_s[j][:], xt_ps[:],
                                             mybir.ActivationFunctionType.Copy)
                        nc.sync.dma_start(xt_dram[j * P:(j + 1) * P, :],
                                          xt_s[j][:])

                # ---------- phase 3: e = xt + xt[nn]  (x2 edge_ft) ----------
                e_aug = [mp.tile([P, NCH], F32, name=f"eaug{j}") for j in range(JT)]
                for j in range(JT):
                    xtg = wp.tile([P, OUT], F32, name="xtg", tag="xtg")
                    nc.gpsimd.indirect_dma_start(
                        out=xtg[:], out_offset=None, in_=xt_dram[:, :],
                        in_offset=IndirectOffsetOnAxis(ap=nnu[j][:, 0:1], axis=0))
                    nc.vector.tensor_add(e_aug[j][:, 0:OUT], xt_s[j][:], xtg[:])
                    nc.vector.memset(e_aug[j][:, OUT:NCH], 1.0)
                    nc.sync.dma_start(e_dram[j * P:(j + 1) * P, :], e_aug[j][:])

                if maxphase < 5:
                    return
                e_self = [mp.tile([P, NCH], F32, name=f"eself{r}")
                          for r in range(MT)]
                for r in range(MT):
                    nc.gpsimd.indirect_dma_start(
                        out=e_self[r][:], out_offset=None, in_=e_dram[:, :],
                        in_offset=IndirectOffsetOnAxis(
                            ap=selfidx_t[:, r:r + 1], axis=0))

                if maxphase < 6:
                    return
                # ---------- phase 4: scatter-add via one-hot matmul ----------
                with tc.tile_pool(name="scatp", bufs=1, space="PSUM") as sp:
                    ns = [sp.tile([P, NCH], F32, name=f"ns{r}") for r in range(MT)]
                    for j in range(JT):
                        oh = wp.tile([P, HALF], F32, name="oh", tag="oh")
                        nc.gpsimd.tensor_scalar(
                            out=oh[:], in0=colidx_t[:], scalar1=nnf[j][:, 0:1],
                            scalar2=None, op0=mybir.AluOpType.is_equal)
                        if sub6 < 2:
                            continue
                        for r in range(MT):
                            nc.tensor.matmul(
                                ns[r][:], lhsT=oh[:, r * P:(r + 1) * P],
                                rhs=e_aug[j][:], start=(j == 0), stop=(j == JT - 1))
                    if sub6 < 3:
                        return

                    # ---------- phase 5: node_ft = (ns + e_self) / (2 deg) ----
                    for r in range(MT):
                        full = wp.tile([P, NCH], F32, name="full", tag="full")
                        nc.vector.tensor_add(full[:], ns[r][:], e_self[r][:])
                        if sub6 < 4:
                            continue
                        deg2 = wp.tile([P, 1], F32, name="deg2", tag="deg2")
                        nc.vector.tensor_scalar_mul(deg2[:], full[:, OUT:NCH], 2.0)
                        rdeg = wp.tile([P, 1], F32, name="rdeg", tag="rdeg")
                        nc.vector.reciprocal(rdeg[:], deg2[:])
                        if sub6 < 5:
                            continue
                        nft = wp.tile([P, OUT], F32, name="nft", tag="nft")
                        nc.vector.tensor_scalar(
                            out=nft[:], in0=full[:, 0:OUT], scalar1=rdeg[:, 0:1],
                            scalar2=None, op0=mybir.AluOpType.mult)
                        if sub6 < 6:
                            continue
                        dst = nf_flat[r * P * OUT:(r + 1) * P * OUT]
                        nc.sync.dma_start(
                            dst.rearrange("(p c) -> p c", p=P, c=OUT), nft[:])

                if maxphase < 7:
                    return
                # ---------- phase 6: BN stats + allreduce ----------
                t0 = mp.tile([P, HALF], F32, name="t0")
                t1 = mp.tile([OUT - P, HALF], F32, name="t1")
                nc.sync.dma_start(
                    t0[:], nf_flat[0:P * HALF].rearrange("(p x) -> p x", p=P,
                                                         x=HALF))
                nc.sync.dma_start(
                    t1[:], nf_flat[P * HALF:FLAT].rearrange(
                        "(p x) -> p x", p=OUT - P, x=HALF))

                sblk_a = mp.tile([P, 2], F32, name="sblk_a")
                sblk_b = mp.tile([OUT - P, 2], F32, name="sblk_b")
                junk0 = mp.tile([P, HALF], F32, name="junk0")
                nc.vector.reduce_sum(sblk_a[:, 0:1], t0[:],
                                     axis=mybir.AxisListType.X)
                nc.vector.scalar_tensor_tensor(
                    out=junk0[:], in0=t0[:], scalar=1.0, in1=t0[:],
                    op0=mybir.AluOpType.mult, op1=mybir.AluOpType.mult,
                    accum_out=sblk_a[:, 1:2])
                nc.vector.reduce_sum(sblk_b[:, 0:1], t1[:],
                                     axis=mybir.AxisListType.X)
                nc.vector.scalar_tensor_tensor(
                    out=junk0[0:OUT - P, :], in0=t1[:], scalar=1.0, in1=t1[:],
                    op0=mybir.AluOpType.mult,
                    op1=mybir.AluOpType.mult, accum_out=sblk_b[:, 1:2])

                b2c_a = mp.tile([P, NCH], F32, name="b2c_a")
                b2c_b = mp.tile([OUT - P, NCH], F32, name="b2c_b")
                nc.sync.dma_start(b2c_a[:], b2c_d[0:P, :])
                nc.sync.dma_start(b2c_b[:], b2c_d[P:OUT, :])

                with tc.tile_pool(name="bnp", bufs=1, space="PSUM") as bp:
                    pst_a = bp.tile([P, 2], F32, name="pst_a")
                    pst_b = bp.tile([NCH - P, 2], F32, name="pst_b")
                    nc.tensor.matmul(pst_a[:], lhsT=b2c_a[:, 0:P], rhs=sblk_a[:],
                                     start=True, stop=False)
                    nc.tensor.matmul(pst_a[:], lhsT=b2c_b[:, 0:P], rhs=sblk_b[:],
                                     start=False, stop=True)
                    nc.tensor.matmul(pst_b[:], lhsT=b2c_a[:, P:NCH], rhs=sblk_a[:],
                                     start=True, stop=False)
                    nc.tensor.matmul(pst_b[:], lhsT=b2c_b[:, P:NCH], rhs=sblk_b[:],
                                     start=False, stop=True)
                    st_a = mp.tile([P, 2], F32, name="st_a")
                    st_b = mp.tile([NCH - P, 2], F32, name="st_b")
                    nc.vector.tensor_copy(st_a[:], pst_a[:])
                    nc.vector.tensor_copy(st_b[:], pst_b[:])
                    nc.sync.dma_start(stats_in[0:P, :], st_a[:])
                    nc.sync.dma_start(stats_in[P:NCH, :], st_b[:])

                    tc.strict_bb_all_engine_barrier()
                    nc.gpsimd.collective_compute(
                        "AllReduce", mybir.AluOpType.add,
                        replica_groups=[[0, 1, 2, 3, 4, 5, 6, 7]],
                        ins=[stats_in.ap().opt()], outs=[stats_out.ap().opt()])
                    tc.strict_bb_all_engine_barrier()

                    ssum_a = mp.tile([P, 2], F32, name="ssum_a")
                    ssum_b = mp.tile([NCH - P, 2], F32, name="ssum_b")
                    nc.sync.dma_start(ssum_a[:], stats_out[0:P, :])
                    nc.sync.dma_start(ssum_b[:], stats_out[P:NCH, :])

                    gam_a = mp.tile([P, 1], F32, name="gam_a")
                    gam_b = mp.tile([NCH - P, 1], F32, name="gam_b")
                    bet_a = mp.tile([P, 1], F32, name="bet_a")
                    bet_b = mp.tile([NCH - P, 1], F32, name="bet_b")
                    nc.sync.dma_start(gam_a[:], gam_d[0:P, None])
                    nc.sync.dma_start(gam_b[:], gam_d[P:NCH, None])
                    nc.sync.dma_start(bet_a[:], bet_d[0:P, None])
                    nc.sync.dma_start(bet_b[:], bet_d[P:NCH, None])

                    def bn_scale_shift(ssum, gam, bet, scsh, rows):
                        mean = mp.tile([rows, 1], F32, name=f"mean{rows}")
                        ex2 = mp.tile([rows, 1], F32, name=f"ex2{rows}")
                        nc.vector.tensor_scalar_mul(mean[:], ssum[:, 0:1],
                                                    1.0 / NELEM)
                        nc.vector.tensor_scalar_mul(ex2[:], ssum[:, 1:2],
                                                    1.0 / NELEM)
                        var = mp.tile([rows, 1], F32, name=f"var{rows}")
                        nc.vector.tensor_tensor(out=var[:], in0=mean[:],
                                                in1=mean[:],
                                                op=mybir.AluOpType.mult)
                        nc.vector.tensor_tensor(out=var[:], in0=ex2[:],
                                                in1=var[:],
                                                op=mybir.AluOpType.subtract)
                        nc.vector.tensor_scalar_add(var[:], var[:], BN_EPS)
                        sd = mp.tile([rows, 1], F32, name=f"sd{rows}")
                        nc.scalar.sqrt(sd[:], var[:])
                        rstd = mp.tile([rows, 1], F32, name=f"rstd{rows}")
                        nc.vector.reciprocal(rstd[:], sd[:])
                        nc.vector.tensor_tensor(out=scsh[:, 0:1], in0=gam[:],
                                                in1=rstd[:],
                                                op=mybir.AluOpType.mult)
                        msc = mp.tile([rows, 1], F32, name=f"msc{rows}")
                        nc.vector.tensor_tensor(out=msc[:], in0=mean[:],
                                                in1=scsh[:, 0:1],
                                                op=mybir.AluOpType.mult)
                        nc.vector.tensor_tensor(out=scsh[:, 1:2], in0=bet[:],
                                                in1=msc[:],
                                                op=mybir.AluOpType.subtract)

                    scsh_a = mp.tile([P, 2], F32, name="scsh_a")
                    scsh_b = mp.tile([NCH - P, 2], F32, name="scsh_b")
                    bn_scale_shift(ssum_a, gam_a, bet_a, scsh_a, P)
                    bn_scale_shift(ssum_b, gam_b, bet_b, scsh_b, NCH - P)

                    c2b_a = mp.tile([P, OUT], F32, name="c2b_a")
                    c2b_b = mp.tile([NCH - P, OUT], F32, name="c2b_b")
                    nc.sync.dma_start(c2b_a[:], c2b_d[0:P, :])
                    nc.sync.dma_start(c2b_b[:], c2b_d[P:NCH, :])

                    pts_a = bp.tile([P, 2], F32, name="pts_a")
                    pts_b = bp.tile([OUT - P, 2], F32, name="pts_b")
                    nc.tensor.matmul(pts_a[:], lhsT=c2b_a[:, 0:P], rhs=scsh_a[:],
                                     start=True, stop=False)
                    nc.tensor.matmul(pts_a[:], lhsT=c2b_b[:, 0:P], rhs=scsh_b[:],
                                     start=False, stop=True)
                    nc.tensor.matmul(pts_b[:], lhsT=c2b_a[:, P:OUT], rhs=scsh_a[:],
                                     start=True, stop=False)
                    nc.tensor.matmul(pts_b[:], lhsT=c2b_b[:, P:OUT], rhs=scsh_b[:],
                                     start=False, stop=True)
                    sct_a = mp.tile([P, 2], F32, name="sct_a")
                    sct_b = mp.tile([OUT - P, 2], F32, name="sct_b")
                    nc.vector.tensor_copy(sct_a[:], pts_a[:])
                    nc.vector.tensor_copy(sct_b[:], pts_b[:])

                    # ---------- phase 7: y = relu(nf * scale + shift) ----------
                    y0 = mp.tile([P, HALF], F32, name="y0")
                    y1 = mp.tile([OUT - P, HALF], F32, name="y1")
                    nc.scalar.activation(y0[:], t0[:],
                                         mybir.ActivationFunctionType.Relu,
                                         bias=sct_a[:, 1:2], scale=sct_a[:, 0:1])
                    nc.scalar.activation(y1[:], t1[:],
                                         mybir.ActivationFunctionType.Relu,
                                         bias=sct_b[:, 1:2], scale=sct_b[:, 0:1])
                    nc.sync.dma_start(y_d[0:P, :], y0[:])
                    nc.sync.dma_start(y_d[P:OUT, :], y1[:])

            _body()

    nc.compile()
    _CACHE["nc"] = nc
    return nc


def _prep_core(x, theta, gamma, beta, b, h):
    xi = np.ascontiguousarray(x[b].reshape(L, C))
    xiT = np.ascontiguousarray(xi.T)
    sq = np.einsum("lc,lc->l", xi, xi, dtype=np.float32)
    sqh = (0.5 * sq).astype(np.float32)

    gsrc = np.zeros((L, GW), dtype=np.float32)
    gsrc[:, 0:C] = xi
    gsrc[:, C] = sqh

    r0 = h * HALF
    colidx = np.broadcast_to(
        (r0 + np.arange(HALF, dtype=np.float32))[None, :], (P, HALF)).copy()
    selfidx = (r0 + np.arange(MT, dtype=np.uint32)[None, :] * P
               + np.arange(P, dtype=np.uint32)[:, None]).astype(np.uint32)
    selfidx = np.ascontiguousarray(selfidx)

    # BN local-block (t) -> channel (c) mapping for this half
    t = np.arange(OUT)
    ch = (h * FLAT + t * HALF) // L
    b2c = np.zeros((OUT, NCH), dtype=np.float32)
    b2c[t, ch] = 1.0
    c2b = np.ascontiguousarray(b2c.T)

    return {
        "xr": xiT,
        "xh": np.ascontiguousarray(xiT[:, r0:r0 + HALF]),
        "x32": xiT,
        "msqh": np.ascontiguousarray((-sqh)[None, :]),
        "onesr": np.ones((1, P), dtype=np.float32),
        "gsrc": gsrc,
        "gself": np.ascontiguousarray(gsrc[r0:r0 + HALF]),
        "theta": np.ascontiguousarray(theta.astype(np.float32)),
        "gamma": np.concatenate([gamma.astype(np.float32),
                                 np.ones(1, np.float32)]),
        "beta": np.concatenate([beta.astype(np.float32),
                                np.zeros(1, np.float32)]),
        "colidx": colidx,
        "selfidx": selfidx,
        "b2c": b2c,
        "c2b": c2b,
    }


def kernel(x, theta, gamma, beta):
    x = np.asarray(x, dtype=np.float32)
    theta = np.asarray(theta, dtype=np.float32)
    gamma = np.asarray(gamma, dtype=np.float32)
    beta = np.asarray(beta, dtype=np.float32)

    nc = _build()
    in_maps = [_prep_core(x, theta, gamma, beta, core // 2, core % 2)
               for core in range(8)]
    trace = bool(int(os.environ.get("KERNEL_TRACE", "0")))
    res = bass_utils.run_bass_kernel_spmd(
        nc, in_maps, core_ids=list(range(8)), trace=trace)

    LAST_INFO["exec_time_ns"] = res.exec_time_ns
    LAST_INFO["trace"] = (res.instructions_and_trace[1]
                          if res.instructions_and_trace else None)
    LAST_INFO["results"] = res.results

    y = np.empty((B, OUT, L, 1), dtype=np.float32)
    for b in range(B):
        flat0 = res.results[2 * b]["y"].reshape(-1)
        flat1 = res.results[2 * b + 1]["y"].reshape(-1)
        y[b] = np.concatenate([flat0, flat1]).reshape(OUT, L, 1)
    return y



# revision 14
# speedup vs baseline: 1.6847x; 1.6847x over previous
"""Trainium2 Bass kernel for nn_DAHH (hypergraph conv + BatchNorm + ReLU).

Sharding: data-parallel over B=4 samples x 2 half-row shards = 8 cores.
Each core owns rows R = [h*1024, (h+1)*1024) of its sample's (2048, 768) node
matrix, finds each row's nearest neighbor (kNN hyperedge construction), and
the pair of cores exchanges nn indices via AllGather. BatchNorm statistics are
combined with an 8-core AllReduce.

Self-contained: hardcodes all shapes; only needs numpy + concourse (bass).
"""

import os
import numpy as np

import concourse.bacc as bacc
import concourse.bass as bass
import concourse.mybir as mybir
import concourse.tile as tile
from concourse import bass_utils
from concourse.bass import IndirectOffsetOnAxis

F32 = mybir.dt.float32
F32R = mybir.dt.float32r
U32 = mybir.dt.uint32

B, C, L, OUT = 4, 768, 2048, 159
P = 128
KT = C // P            # 6 k-tiles
HALF = L // 2          # 1024 rows per core
MT = HALF // P         # 8 m-tiles per core (own rows)
JT = L // P            # 16 j-tiles (all rows)
GW = C + 4             # gather row width (768 xi + sq/2 + pad)
NCH = OUT + 1          # 160: padded channel dim (col 159 = ones/deg)
FLAT = HALF * OUT      # 162816 = 159 * 1024 flat elements per core
NCAND = 3              # nn candidates refined exactly (from fp32r top-8)
BN_EPS = 1e-5
NELEM = float(B * L)   # elements per BN channel across batch

LAST_INFO = {}

_CACHE = {}


class _Done(Exception):
    pass


def _build():
    if "nc" in _CACHE:
        return _CACHE["nc"]
    maxphase = int(os.environ.get("KERNEL_MAXPHASE", "99"))
    subp = int(os.environ.get("KERNEL_SUBP", "9"))
    sub6 = int(os.environ.get("KERNEL_SUB6", "9"))

    nc = bacc.Bacc("TRN2", target_bir_lowering=False, debug=False,
                   num_devices=8)

    # ---- DRAM I/O (per-core contents differ, shapes uniform) ----
    xr_d = nc.dram_tensor("xr", [C, L], F32R, kind="ExternalInput")
    xh_d = nc.dram_tensor("xh", [C, HALF], F32R, kind="ExternalInput")
    msqh_d = nc.dram_tensor("msqh", [1, L], F32R, kind="ExternalInput")
    onesr_d = nc.dram_tensor("onesr", [1, P], F32R, kind="ExternalInput")
    gsrc_d = nc.dram_tensor("gsrc", [L, GW], F32, kind="ExternalInput")
    gself_d = nc.dram_tensor("gself", [HALF, GW], F32, kind="ExternalInput")
    th_d = nc.dram_tensor("theta", [C, 256], F32R, kind="ExternalInput")
    gam_d = nc.dram_tensor("gamma", [NCH], F32, kind="ExternalInput")
    bet_d = nc.dram_tensor("beta", [NCH], F32, kind="ExternalInput")
    colidx_d = nc.dram_tensor("colidx", [P, HALF], F32, kind="ExternalInput")
    econst_d = nc.dram_tensor("econst", [P, 97], F32R, kind="ExternalInput")
    selfidx_d = nc.dram_tensor("selfidx", [P, MT], U32, kind="ExternalInput")
    b2c_d = nc.dram_tensor("b2c", [OUT, NCH], F32, kind="ExternalInput")
    c2b_d = nc.dram_tensor("c2b", [NCH, OUT], F32, kind="ExternalInput")

    y_d = nc.dram_tensor("y", [OUT, HALF], F32, kind="ExternalOutput")
    nn_out_d = nc.dram_tensor("nn_out", [L, 1], U32, kind="ExternalOutput")

    # DRAM scratch
    xt_dram = nc.dram_tensor("xt_scr", [L, OUT], F32)
    e_dram = nc.dram_tensor("e_scr", [L, NCH], F32R)
    nn_half = nc.dram_tensor("nn_half", [HALF, 1], U32)
    nn_full = nc.dram_tensor("nn_full", [L, 1], U32)
    nf_flat = nc.dram_tensor("nf_flat", [FLAT], F32)
    stats_in = nc.dram_tensor("stats_in", [NCH, 2], F32)
    stats_all = nc.dram_tensor("stats_all", [8 * NCH * 2], F32)

    with tile.TileContext(nc) as tc:
        with (
            tc.tile_pool(name="main", bufs=1) as mp,
            tc.tile_pool(name="work", bufs=2) as wp,
        ):
            def _body():
                # ---------- persistent loads ----------
                xr_t = [mp.tile([P, L], F32R, name=f"xr{k}") for k in range(KT)]
                xh_t = [mp.tile([P, HALF], F32R, name=f"xh{k}") for k in range(KT)]
                th_t = [mp.tile([P, 256], F32R, name=f"th{k}") for k in range(KT)]
                for k in range(KT):
                    nc.sync.dma_start(xh_t[k][:], xh_d[k * P:(k + 1) * P, :])
                    nc.sync.dma_start(xr_t[k][:], xr_d[k * P:(k + 1) * P, :])
                    nc.sync.dma_start(th_t[k][:], th_d[k * P:(k + 1) * P, :])
                msqh_t = mp.tile([1, L], F32R, name="msqh_t")
                nc.sync.dma_start(msqh_t[:], msqh_d[:, :])
                onesr_t = mp.tile([1, P], F32R, name="onesr_t")
                nc.sync.dma_start(onesr_t[:], onesr_d[:, :])
                colidx_t = mp.tile([P, HALF], F32, name="colidx_t")
                nc.sync.dma_start(colidx_t[:], colidx_d[:, :])
                selfidx_t = mp.tile([P, MT], U32, name="selfidx_t")
                nc.sync.dma_start(selfidx_t[:], selfidx_d[:, :])

                # ---------- phase 1: Gram (fp32r) + top-8 + exact refine ----------
                with tc.tile_pool(name="gramp", bufs=2, space="PSUM") as gp:
                    for m in range(MT):
                        g_ps = gp.tile([P, L], F32, name="g_ps", tag="g")
                        lhs = [xh_t[k][:, m * P:(m + 1) * P] for k in range(KT)]
                        for k in range(KT):
                            for chk in range(L // 512):
                                nc.tensor.matmul(
                                    g_ps[:, chk * 512:(chk + 1) * 512],
                                    lhsT=lhs[k],
                                    rhs=xr_t[k][:, chk * 512:(chk + 1) * 512],
                                    start=(k == 0), stop=False)
                        for chk in range(L // 512):
                            nc.tensor.matmul(
                                g_ps[:, chk * 512:(chk + 1) * 512],
                                lhsT=onesr_t[:, :],
                                rhs=msqh_t[:, chk * 512:(chk + 1) * 512],
                                start=False, stop=True)

                        mneg = wp.tile([P, L], F32, name="mneg", tag="mneg")
                        nc.scalar.activation(mneg[:], g_ps[:],
                                             mybir.ActivationFunctionType.Copy)

                        mx8 = wp.tile([P, 8], F32, name="mx8", tag="mx8")
                        idx8 = wp.tile([P, 8], U32, name="idx8", tag="idx8")
                        nc.vector.max(out=mx8[:], in_=mneg[:])
                        nc.vector.max_index(out=idx8[:], in_max=mx8[:],
                                            in_values=mneg[:])

                        if maxphase < 2:
                            nc.sync.dma_start(nn_half[m * P:(m + 1) * P, :],
                                              idx8[:, 1:2])
                            continue
                        # exact fp32 refine of candidates 1..NCAND
                        gs = wp.tile([P, GW], F32, name="gs", tag="gs")
                        nc.sync.dma_start(gs[:], gself_d[m * P:(m + 1) * P, :])
                        mc_list = []
                        for c in range(1, NCAND + 1):
                            xg = wp.tile([P, GW], F32, name=f"xg{c}", tag="xg",
                                         bufs=3)
                            nc.gpsimd.indirect_dma_start(
                                out=xg[:], out_offset=None,
                                in_=gsrc_d[:, :],
                                in_offset=IndirectOffsetOnAxis(
                                    ap=idx8[:, c:c + 1], axis=0))
                            if subp < 2:
                                continue
                            junk = wp.tile([P, C], F32, name="junk", tag="junk")
                            mdot = wp.tile([P, 1], F32, name=f"mdot{c}",
                                           tag=f"mdot{c}")
                            nc.vector.scalar_tensor_tensor(
                                out=junk[:], in0=gs[:, 0:C], scalar=1.0,
                                in1=xg[:, 0:C],
                                op0=mybir.AluOpType.mult,
                                op1=mybir.AluOpType.mult,
                                accum_out=mdot[:])
                            if subp < 3:
                                continue
                            mc = wp.tile([P, 1], F32, name=f"mc{c}", tag=f"mc{c}")
                            # mc = dot - sq[j]/2  (exact stage-2 score)
                            nc.vector.scalar_tensor_tensor(
                                out=mc[:], in0=mdot[:], scalar=1.0,
                                in1=xg[:, C:C + 1],
                                op0=mybir.AluOpType.mult,
                                op1=mybir.AluOpType.subtract)
                            mc_list.append(mc)
                        if subp < 4:
                            nc.sync.dma_start(nn_half[m * P:(m + 1) * P, :],
                                              idx8[:, 1:2])
                            continue

                        bestm = wp.tile([P, 1], F32, name="bestm", tag="bestm")
                        besti = wp.tile([P, 1], U32, name="besti", tag="besti")
                        nc.vector.tensor_copy(bestm[:], mc_list[0][:])
                        nc.vector.tensor_copy(besti[:], idx8[:, 1:2])
                        for c in range(2, NCAND + 1):
                            mask = wp.tile([P, 1], U32, name=f"mask{c}",
                                           tag=f"mask{c}")
                            nc.vector.tensor_tensor(
                                out=mask[:], in0=mc_list[c - 1][:], in1=bestm[:],
                                op=mybir.AluOpType.is_gt)
                            nc.vector.copy_predicated(bestm[:], mask[:],
                                                      mc_list[c - 1][:])
                            nc.vector.copy_predicated(besti[:], mask[:],
                                                      idx8[:, c:c + 1])
                        nc.sync.dma_start(nn_half[m * P:(m + 1) * P, :], besti[:])

                # ---------- phase 2: xt = xi @ theta (f32r, from SBUF) ----------
                # nn-independent: runs before the collective so the PE stays
                # busy while nn_half lands in DRAM.
                xt_s = [mp.tile([P, OUT], F32, name=f"xts{j}") for j in range(JT)]
                with tc.tile_pool(name="xtp", bufs=2, space="PSUM") as xp:
                    for j in range(JT):
                        xt_ps = xp.tile([P, 256], F32, name="xt_ps", tag="xtps")
                        for k in range(KT):
                            nc.tensor.matmul(
                                xt_ps[:], lhsT=xr_t[k][:, j * P:(j + 1) * P],
                                rhs=th_t[k][:], start=(k == 0), stop=(k == KT - 1))
                        nc.scalar.activation(xt_s[j][:], xt_ps[:, 0:OUT],
                                             mybir.ActivationFunctionType.Copy)
                        nc.sync.dma_start(xt_dram[j * P:(j + 1) * P, :],
                                          xt_s[j][:])

                # ---------- nn allgather within the sample pair ----------
                if maxphase < 3:
                    nc.sync.dma_start(nn_out_d[0:HALF, :], nn_half[:, :])
                    return
                tc.strict_bb_all_engine_barrier()
                nc.gpsimd.collective_compute(
                    "AllGather", mybir.AluOpType.bypass,
                    replica_groups=[[0, 1], [2, 3], [4, 5], [6, 7]],
                    ins=[nn_half.ap().opt()], outs=[nn_full.ap().opt()])
                tc.strict_bb_all_engine_barrier()
                nc.sync.dma_start(nn_out_d[:, :], nn_full[:, :])

                nnu = [mp.tile([P, 1], U32, name=f"nnu{j}") for j in range(JT)]
                nnf = [mp.tile([P, 1], F32, name=f"nnf{j}") for j in range(JT)]
                for j in range(JT):
                    nc.sync.dma_start(nnu[j][:], nn_full[j * P:(j + 1) * P, :])
                    nc.gpsimd.tensor_copy(nnf[j][:], nnu[j][:])

                if maxphase < 4:
                    return

                # ---------- phase 3: e = xt + xt[nn]  (x2 edge_ft) ----------
                e_aug = [mp.tile([P, 256], F32R, name=f"eaug{j}") for j in range(JT)]
                for j in range(JT):
                    nc.scalar.dma_start(e_aug[j][:, OUT:256], econst_d[:, :])
                for j in range(JT):
                    xtg = wp.tile([P, OUT], F32, name="xtg", tag="xtg")
                    nc.gpsimd.indirect_dma_start(
                        out=xtg[:], out_offset=None, in_=xt_dram[:, :],
                        in_offset=IndirectOffsetOnAxis(ap=nnu[j][:, 0:1], axis=0))
                    nc.vector.tensor_add(e_aug[j][:, 0:OUT], xt_s[j][:], xtg[:])
                    nc.sync.dma_start(e_dram[j * P:(j + 1) * P, :],
                                      e_aug[j][:, 0:NCH])

                if maxphase < 5:
                    return
                e_self = [mp.tile([P, NCH], F32R, name=f"eself{r}")
                          for r in range(MT)]
                for r in range(MT):
                    nc.gpsimd.indirect_dma_start(
                        out=e_self[r][:], out_offset=None, in_=e_dram[:, :],
                        in_offset=IndirectOffsetOnAxis(
                            ap=selfidx_t[:, r:r + 1], axis=0))

                if maxphase < 6:
                    return
                # ---------- phase 4: scatter-add via one-hot matmul ----------
                with tc.tile_pool(name="scatp", bufs=1, space="PSUM") as sp:
                    ns = [sp.tile([P, 256], F32, name=f"ns{r}") for r in range(MT)]
                    for j in range(JT):
                        oh = wp.tile([P, HALF], F32R, name="oh", tag="oh",
                                     bufs=3)
                        nc.vector.tensor_scalar(
                            out=oh[:], in0=colidx_t[:], scalar1=nnf[j][:, 0:1],
                            scalar2=None, op0=mybir.AluOpType.is_equal)
                        if sub6 < 2:
                            continue
                        for r in range(MT):
                            nc.tensor.matmul(
                                ns[r][:],
                                lhsT=oh[:, r * P:(r + 1) * P],
                                rhs=e_aug[j][:],
                                start=(j == 0), stop=(j == JT - 1))
                    if sub6 < 3:
                        return

                    # ---------- phase 5: node_ft = (ns + e_self) / (2 deg) ----
                    for r in range(MT):
                        full = wp.tile([P, NCH], F32, name="full", tag="full")
                        nc.vector.tensor_add(full[:], ns[r][:, 0:NCH],
                                             e_self[r][:].bitcast(F32))
                        if sub6 < 4:
                            continue
                        deg2 = wp.tile([P, 1], F32, name="deg2", tag="deg2")
                        nc.vector.tensor_scalar_mul(deg2[:], full[:, OUT:NCH], 2.0)
                        rdeg = wp.tile([P, 1], F32, name="rdeg", tag="rdeg")
                        nc.vector.reciprocal(rdeg[:], deg2[:])
                        if sub6 < 5:
                            continue
                        nft = wp.tile([P, OUT], F32, name="nft", tag="nft")
                        nc.vector.tensor_scalar(
                            out=nft[:], in0=full[:, 0:OUT], scalar1=rdeg[:, 0:1],
                            scalar2=None, op0=mybir.AluOpType.mult)
                        if sub6 < 6:
                            continue
                        dst = nf_flat[r * P * OUT:(r + 1) * P * OUT]
                        nc.sync.dma_start(
                            dst.rearrange("(p c) -> p c", p=P, c=OUT), nft[:])

                if maxphase < 7:
                    return
                # ---------- phase 6: BN stats + allreduce ----------
                t0 = mp.tile([P, HALF], F32, name="t0")
                t1 = mp.tile([OUT - P, HALF], F32, name="t1")
                nc.sync.dma_start(
                    t0[:], nf_flat[0:P * HALF].rearrange("(p x) -> p x", p=P,
                                                         x=HALF))
                nc.sync.dma_start(
                    t1[:], nf_flat[P * HALF:FLAT].rearrange(
                        "(p x) -> p x", p=OUT - P, x=HALF))

                sblk_a = mp.tile([P, 2], F32, name="sblk_a")
                sblk_b = mp.tile([OUT - P, 2], F32, name="sblk_b")
                junk0 = mp.tile([P, HALF], F32, name="junk0")
                nc.vector.reduce_sum(sblk_a[:, 0:1], t0[:],
                                     axis=mybir.AxisListType.X)
                nc.vector.scalar_tensor_tensor(
                    out=junk0[:], in0=t0[:], scalar=1.0, in1=t0[:],
                    op0=mybir.AluOpType.mult, op1=mybir.AluOpType.mult,
                    accum_out=sblk_a[:, 1:2])
                nc.vector.reduce_sum(sblk_b[:, 0:1], t1[:],
                                     axis=mybir.AxisListType.X)
                nc.vector.scalar_tensor_tensor(
                    out=junk0[0:OUT - P, :], in0=t1[:], scalar=1.0, in1=t1[:],
                    op0=mybir.AluOpType.mult,
                    op1=mybir.AluOpType.mult, accum_out=sblk_b[:, 1:2])

                b2c_a = mp.tile([P, NCH], F32, name="b2c_a")
                b2c_b = mp.tile([OUT - P, NCH], F32, name="b2c_b")
                nc.sync.dma_start(b2c_a[:], b2c_d[0:P, :])
                nc.sync.dma_start(b2c_b[:], b2c_d[P:OUT, :])

                with tc.tile_pool(name="bnp", bufs=1, space="PSUM") as bp:
                    pst_a = bp.tile([P, 2], F32, name="pst_a")
                    pst_b = bp.tile([NCH - P, 2], F32, name="pst_b")
                    nc.tensor.matmul(pst_a[:], lhsT=b2c_a[:, 0:P], rhs=sblk_a[:],
                                     start=True, stop=False)
                    nc.tensor.matmul(pst_a[:], lhsT=b2c_b[:, 0:P], rhs=sblk_b[:],
                                     start=False, stop=True)
                    nc.tensor.matmul(pst_b[:], lhsT=b2c_a[:, P:NCH], rhs=sblk_a[:],
                                     start=True, stop=False)
                    nc.tensor.matmul(pst_b[:], lhsT=b2c_b[:, P:NCH], rhs=sblk_b[:],
                                     start=False, stop=True)
                    st_a = mp.tile([P, 2], F32, name="st_a")
                    st_b = mp.tile([NCH - P, 2], F32, name="st_b")
                    nc.vector.tensor_copy(st_a[:], pst_a[:])
                    nc.vector.tensor_copy(st_b[:], pst_b[:])
                    nc.sync.dma_start(stats_in[0:P, :], st_a[:])
                    nc.sync.dma_start(stats_in[P:NCH, :], st_b[:])

                    tc.strict_bb_all_engine_barrier()
                    nc.gpsimd.collective_compute(
                        "AllGather", mybir.AluOpType.bypass,
                        replica_groups=[[0, 1, 2, 3, 4, 5, 6, 7]],
                        ins=[stats_in.ap().opt()], outs=[stats_all.ap().opt()])
                    tc.strict_bb_all_engine_barrier()

                    # local 8-way reduce of gathered stats (cheaper than the
                    # CC AllReduce program)
                    sa = [mp.tile([P, 2], F32, name=f"sa{c}") for c in range(8)]
                    sb = [mp.tile([NCH - P, 2], F32, name=f"sb{c}")
                          for c in range(8)]
                    for c in range(8):
                        base = c * NCH * 2
                        nc.sync.dma_start(
                            sa[c][:],
                            stats_all[base:base + 2 * P].rearrange(
                                "(p s) -> p s", p=P, s=2))
                        nc.sync.dma_start(
                            sb[c][:],
                            stats_all[base + 2 * P:base + 2 * NCH].rearrange(
                                "(p s) -> p s", p=NCH - P, s=2))

                    def tree_reduce(tiles, rows, tag):
                        lvl = tiles
                        li = 0
                        while len(lvl) > 1:
                            nxt = []
                            for i in range(0, len(lvl), 2):
                                t = mp.tile([rows, 2], F32,
                                            name=f"tr{tag}_{li}_{i}")
                                nc.vector.tensor_add(t[:], lvl[i][:],
                                                     lvl[i + 1][:])
                                nxt.append(t)
                            lvl = nxt
                            li += 1
                        return lvl[0]

                    ssum_a = tree_reduce(sa, P, "a")
                    ssum_b = tree_reduce(sb, NCH - P, "b")

                    gam_a = mp.tile([P, 1], F32, name="gam_a")
                    gam_b = mp.tile([NCH - P, 1], F32, name="gam_b")
                    bet_a = mp.tile([P, 1], F32, name="bet_a")
                    bet_b = mp.tile([NCH - P, 1], F32, name="bet_b")
                    nc.sync.dma_start(gam_a[:], gam_d[0:P, None])
                    nc.sync.dma_start(gam_b[:], gam_d[P:NCH, None])
                    nc.sync.dma_start(bet_a[:], bet_d[0:P, None])
                    nc.sync.dma_start(bet_b[:], bet_d[P:NCH, None])

                    def bn_scale_shift(ssum, gam, bet, scsh, rows):
                        mean = mp.tile([rows, 1], F32, name=f"mean{rows}")
                        ex2 = mp.tile([rows, 1], F32, name=f"ex2{rows}")
                        nc.vector.tensor_scalar_mul(mean[:], ssum[:, 0:1],
                                                    1.0 / NELEM)
                        nc.vector.tensor_scalar_mul(ex2[:], ssum[:, 1:2],
                                                    1.0 / NELEM)
                        var = mp.tile([rows, 1], F32, name=f"var{rows}")
                        nc.vector.tensor_tensor(out=var[:], in0=mean[:],
                                                in1=mean[:],
                                                op=mybir.AluOpType.mult)
                        nc.vector.tensor_tensor(out=var[:], in0=ex2[:],
                                                in1=var[:],
                                                op=mybir.AluOpType.subtract)
                        nc.vector.tensor_scalar_add(var[:], var[:], BN_EPS)
                        sd = mp.tile([rows, 1], F32, name=f"sd{rows}")
                        nc.scalar.sqrt(sd[:], var[:])
                        rstd = mp.tile([rows, 1], F32, name=f"rstd{rows}")
                        nc.vector.reciprocal(rstd[:], sd[:])
                        nc.vector.tensor_tensor(out=scsh[:, 0:1], in0=gam[:],
                                                in1=rstd[:],
                                                op=mybir.AluOpType.mult)
                        msc = mp.tile([rows, 1], F32, name=f"msc{rows}")
                        nc.vector.tensor_tensor(out=msc[:], in0=mean[:],
                                                in1=scsh[:, 0:1],
                                                op=mybir.AluOpType.mult)
                        nc.vector.tensor_tensor(out=scsh[:, 1:2], in0=bet[:],
                                                in1=msc[:],
                                                op=mybir.AluOpType.subtract)

                    scsh_a = mp.tile([P, 2], F32, name="scsh_a")
                    scsh_b = mp.tile([NCH - P, 2], F32, name="scsh_b")
                    bn_scale_shift(ssum_a, gam_a, bet_a, scsh_a, P)
                    bn_scale_shift(ssum_b, gam_b, bet_b, scsh_b, NCH - P)

                    c2b_a = mp.tile([P, OUT], F32, name="c2b_a")
                    c2b_b = mp.tile([NCH - P, OUT], F32, name="c2b_b")
                    nc.sync.dma_start(c2b_a[:], c2b_d[0:P, :])
                    nc.sync.dma_start(c2b_b[:], c2b_d[P:NCH, :])

                    pts_a = bp.tile([P, 2], F32, name="pts_a")
                    pts_b = bp.tile([OUT - P, 2], F32, name="pts_b")
                    nc.tensor.matmul(pts_a[:], lhsT=c2b_a[:, 0:P], rhs=scsh_a[:],
                                     start=True, stop=False)
                    nc.tensor.matmul(pts_a[:], lhsT=c2b_b[:, 0:P], rhs=scsh_b[:],
                                     start=False, stop=True)
                    nc.tensor.matmul(pts_b[:], lhsT=c2b_a[:, P:OUT], rhs=scsh_a[:],
                                     start=True, stop=False)
                    nc.tensor.matmul(pts_b[:], lhsT=c2b_b[:, P:OUT], rhs=scsh_b[:],
                                     start=False, stop=True)
                    sct_a = mp.tile([P, 2], F32, name="sct_a")
                    sct_b = mp.tile([OUT - P, 2], F32, name="sct_b")
                    nc.vector.tensor_copy(sct_a[:], pts_a[:])
                    nc.vector.tensor_copy(sct_b[:], pts_b[:])

                    # ---------- phase 7: y = relu(nf * scale + shift) ----------
                    y0 = mp.tile([P, HALF], F32, name="y0")
                    y1 = mp.tile([OUT - P, HALF], F32, name="y1")
                    nc.scalar.activation(y0[:], t0[:],
                                         mybir.ActivationFunctionType.Relu,
                                         bias=sct_a[:, 1:2], scale=sct_a[:, 0:1])
                    nc.scalar.activation(y1[:], t1[:],
                                         mybir.ActivationFunctionType.Relu,
                                         bias=sct_b[:, 1:2], scale=sct_b[:, 0:1])
                    nc.sync.dma_start(y_d[0:P, :], y0[:])
                    nc.sync.dma_start(y_d[P:OUT, :], y1[:])

            _body()

    nc.compile()
    _CACHE["nc"] = nc
    return nc


def _prep_core(x, theta, gamma, beta, b, h):
    xi = np.ascontiguousarray(x[b].reshape(L, C))
    xiT = np.ascontiguousarray(xi.T)
    sq = np.einsum("lc,lc->l", xi, xi, dtype=np.float32)
    sqh = (0.5 * sq).astype(np.float32)

    gsrc = np.zeros((L, GW), dtype=np.float32)
    gsrc[:, 0:C] = xi
    gsrc[:, C] = sqh

    r0 = h * HALF
    colidx = np.broadcast_to(
        (r0 + np.arange(HALF, dtype=np.float32))[None, :], (P, HALF)).copy()
    selfidx = (r0 + np.arange(MT, dtype=np.uint32)[None, :] * P
               + np.arange(P, dtype=np.uint32)[:, None]).astype(np.uint32)
    selfidx = np.ascontiguousarray(selfidx)

    # BN local-block (t) -> channel (c) mapping for this half
    t = np.arange(OUT)
    ch = (h * FLAT + t * HALF) // L
    b2c = np.zeros((OUT, NCH), dtype=np.float32)
    b2c[t, ch] = 1.0
    c2b = np.ascontiguousarray(b2c.T)

    thp = np.zeros((C, 256), dtype=np.float32)
    thp[:, 0:OUT] = theta.astype(np.float32)

    return {
        "xr": xiT,
        "xh": np.ascontiguousarray(xiT[:, r0:r0 + HALF]),
        "msqh": np.ascontiguousarray((-sqh)[None, :]),
        "onesr": np.ones((1, P), dtype=np.float32),
        "gsrc": gsrc,
        "gself": np.ascontiguousarray(gsrc[r0:r0 + HALF]),
        "theta": thp,
        "gamma": np.concatenate([gamma.astype(np.float32),
                                 np.ones(1, np.float32)]),
        "beta": np.concatenate([beta.astype(np.float32),
                                np.zeros(1, np.float32)]),
        "colidx": colidx,
        "econst": np.concatenate([np.ones((P, 1), np.float32),
                                  np.zeros((P, 96), np.float32)], axis=1),
        "selfidx": selfidx,
        "b2c": b2c,
        "c2b": c2b,
    }


def kernel(x, theta, gamma, beta):
    x = np.asarray(x, dtype=np.float32)
    theta = np.asarray(theta, dtype=np.float32)
    gamma = np.asarray(gamma, dtype=np.float32)
    beta = np.asarray(beta, dtype=np.float32)

    nc = _build()
    in_maps = [_prep_core(x, theta, gamma, beta, core // 2, core % 2)
               for core in range(8)]
    trace = bool(int(os.environ.get("KERNEL_TRACE", "0")))
    res = bass_utils.run_bass_kernel_spmd(
        nc, in_maps, core_ids=list(range(8)), trace=trace)

    LAST_INFO["exec_time_ns"] = res.exec_time_ns
    LAST_INFO["trace"] = (res.instructions_and_trace[1]
                          if res.instructions_and_trace else None)
    LAST_INFO["results"] = res.results

    y = np.empty((B, OUT, L, 1), dtype=np.float32)
    for b in range(B):
        flat0 = res.results[2 * b]["y"].reshape(-1)
        flat1 = res.results[2 * b + 1]["y"].reshape(-1)
        y[b] = np.concatenate([flat0, flat1]).reshape(OUT, L, 1)
    return y

